# revision 4
# baseline (speedup 1.0000x reference)
"""Squeeze-and-Excitation gate kernel for Trainium2 (Bass/Tile).

Reference computation (per sample b):
    s = mean(x[b], axis=(H, W))                # [C]
    h = relu(w1 @ s + b1)                      # [Cr]
    g = sigmoid(w2 @ h + b2)                   # [C]
    out[b] = x[b] * g[:, None, None]

Sharding: data-parallel over batch across 8 NeuronCores (8 samples each),
gate weights replicated. Each core streams each sample through SBUF once
(minimum 1x read + 1x write of x).

Perf notes (from NTFF profile analysis):
  - All x loads AND stores share the single SP HWDGE ring: with one
    FIFO ring the SDMA engines alternate reads/writes at whole-sample
    granularity and sustain ~405 GB/s; splitting loads/stores onto the
    SP+ACT rings made engines interleave R/W per ~2KB packet and
    dropped the wire to ~353 GB/s (measured).
  - Head-of-line blocking is avoided by issue order alone: sample b's
    stores are issued AFTER sample b+1's loads, so their semaphores
    (scale-mul done) are long ready when they reach the ring head, and
    the last sample's gate latency is covered by the previous sample's
    stores draining.
  - Weight matrices are pre-transposed on the host into matmul-ready
    lhsT layouts (no on-device PE-transpose preamble) and loaded via
    the gpsimd SWDGE ring so SP's first instruction is an x load.
  - The scale-multiply alternates DVE/ACT to halve gate-apply latency.
"""

import contextlib

import numpy as np

import concourse.bacc as bacc
import concourse.mybir as mybir
import concourse.tile as tile
from concourse import bass_utils

N_CORES = 8
B, C, H, W = 64, 512, 56, 56
HW = H * W              # 3136
BL = B // N_CORES       # 8 local samples per core
P = 128                 # SBUF partitions
NCH = C // P            # 4 channel chunks of 128
R = 32                  # squeezed channels (Cr)
INV_HW = 1.0 / HW

_CACHE = {}
LAST_RESULTS = None     # test harness reads trace/exec info from here


def _emit(ctx, tc, out, x, w1t, b1t, w2t, b2t):
    nc = tc.nc
    f32 = mybir.dt.float32

    singles = ctx.enter_context(tc.tile_pool(name="singles", bufs=1))
    xpool = ctx.enter_context(tc.tile_pool(name="xpool", bufs=12))
    spool = ctx.enter_context(tc.tile_pool(name="spool", bufs=4))
    hpool = ctx.enter_context(tc.tile_pool(name="hpool", bufs=4))
    gpool = ctx.enter_context(tc.tile_pool(name="gpool", bufs=4))
    pp_h = ctx.enter_context(tc.tile_pool(name="pp_h", bufs=2, space="PSUM"))
    pp_g = ctx.enter_context(tc.tile_pool(name="pp_g", bufs=2, space="PSUM"))

    # ---- weights/biases arrive in matmul-ready layouts; load off the SP
    # ring (gpsimd/SWDGE) so the first x load is SP's first instruction.
    w1s = singles.tile([P, NCH, R], f32)    # w1s[p,t,r] = w1[r,t*128+p]/HW
    nc.gpsimd.dma_start(out=w1s, in_=w1t)
    w2s = singles.tile([R, NCH, P], f32)    # w2s[r,t,p] = w2[t*128+p,r]
    nc.gpsimd.dma_start(out=w2s, in_=w2t)
    b1s = singles.tile([R, 1], f32)
    nc.gpsimd.dma_start(out=b1s, in_=b1t)
    b2s = singles.tile([P, NCH], f32)       # b2s[p,t] = b2[t*128+p]
    nc.gpsimd.dma_start(out=b2s, in_=b2t)

    def store(b_idx, t, xt):
        nc.sync.dma_start(out=out[b_idx, t * P:(t + 1) * P, :], in_=xt)

    # ---- main loop over local samples; sample b's stores are issued
    # after sample b+1's loads (software-pipelined issue order) ----
    prev = None
    for b in range(BL):
        s = spool.tile([P, NCH], f32)                # pooled sums per chunk
        xts = []
        for t in range(NCH):
            xt = xpool.tile([P, HW], f32, tag="x")
            nc.sync.dma_start(out=xt, in_=x[b, t * P:(t + 1) * P, :])
            nc.vector.reduce_sum(s[:, t:t + 1], xt, axis=mybir.AxisListType.X)
            xts.append(xt)

        if prev is not None:
            pb, pxts = prev
            for t in range(NCH):
                store(pb, t, pxts[t])

        # h = relu(w1 @ mean + b1): accumulate over the 4 channel chunks
        ph = pp_h.tile([R, 1], f32)
        for t in range(NCH):
            nc.tensor.matmul(ph, w1s[:, t, :], s[:, t:t + 1],
                             start=(t == 0), stop=(t == NCH - 1))
        h = hpool.tile([R, 1], f32)
        nc.vector.tensor_scalar(out=h, in0=ph, scalar1=b1s, scalar2=0.0,
                                op0=mybir.AluOpType.add, op1=mybir.AluOpType.max)

        # g[t] = sigmoid(w2[t] @ h + b2[t])
        pg = pp_g.tile([P, NCH], f32)
        g = gpool.tile([P, NCH], f32)
        for t in range(NCH):
            nc.tensor.matmul(pg[:, t:t + 1], w2s[:, t, :], h, start=True, stop=True)
            nc.scalar.activation(g[:, t:t + 1], pg[:, t:t + 1],
                                 mybir.ActivationFunctionType.Sigmoid,
                                 bias=b2s[:, t:t + 1], scale=1.0)

        # scale (DVE and ACT alternate; stores issue next iteration)
        nc.vector.tensor_scalar_mul(xts[0], xts[0], g[:, 0:1])
        nc.scalar.mul(xts[1], xts[1], g[:, 1:2])
        nc.vector.tensor_scalar_mul(xts[2], xts[2], g[:, 2:3])
        nc.scalar.mul(xts[3], xts[3], g[:, 3:4])
        prev = (b, xts)

    # flush the last sample's stores
    pb, pxts = prev
    for t in range(NCH):
        store(pb, t, pxts[t])


def _build():
    f32 = mybir.dt.float32
    nc = bacc.Bacc("TRN2", target_bir_lowering=False, debug=False,
                   num_devices=N_CORES)
    x = nc.dram_tensor("x", [BL, C, HW], f32, kind="ExternalInput").ap()
    w1t = nc.dram_tensor("w1t", [P, NCH, R], f32, kind="ExternalInput").ap()
    b1t = nc.dram_tensor("b1t", [R, 1], f32, kind="ExternalInput").ap()
    w2t = nc.dram_tensor("w2t", [R, NCH, P], f32, kind="ExternalInput").ap()
    b2t = nc.dram_tensor("b2t", [P, NCH], f32, kind="ExternalInput").ap()
    out = nc.dram_tensor("out", [BL, C, HW], f32, kind="ExternalOutput").ap()

    with tile.TileContext(nc) as tc:
        with contextlib.ExitStack() as ctx:
            _emit(ctx, tc, out, x, w1t, b1t, w2t, b2t)
    nc.compile()
    return nc


def _get_module():
    if "nc" not in _CACHE:
        _CACHE["nc"] = _build()
    return _CACHE["nc"]


def kernel(**inputs):
    global LAST_RESULTS
    x = np.ascontiguousarray(inputs["x"], dtype=np.float32)
    w1 = np.asarray(inputs["w1"], dtype=np.float32)
    b1 = np.asarray(inputs["b1"], dtype=np.float32)
    w2 = np.asarray(inputs["w2"], dtype=np.float32)
    b2 = np.asarray(inputs["b2"], dtype=np.float32)

    # matmul-ready lhsT layouts (see _emit)
    w1t = np.ascontiguousarray(
        (w1.T * INV_HW).reshape(NCH, P, R).transpose(1, 0, 2))
    w2t = np.ascontiguousarray(w2.reshape(NCH, P, R).transpose(2, 0, 1))
    b1t = np.ascontiguousarray(b1.reshape(R, 1))
    b2t = np.ascontiguousarray(b2.reshape(NCH, P).T)

    nc = _get_module()
    xr = x.reshape(B, C, HW)
    in_maps = [
        {
            "x": xr[i * BL:(i + 1) * BL],
            "w1t": w1t,
            "b1t": b1t,
            "w2t": w2t,
            "b2t": b2t,
        }
        for i in range(N_CORES)
    ]
    res = bass_utils.run_bass_kernel_spmd(
        nc, in_maps, core_ids=list(range(N_CORES))
    )
    LAST_RESULTS = res
    out = np.concatenate([res.results[i]["out"] for i in range(N_CORES)], axis=0)
    return out.reshape(B, C, H, W)
